# revision 10
# baseline (speedup 1.0000x reference)
"""Causal single-head attention on 8 TRN2 NeuronCores (Bass/Tile).

Problem: x[4,4096,1024] @ {Wq,Wk,Wv}[1024,64] (+zero biases) -> causal
softmax attention -> out[4,4096,64], fp32.

Sharding: 8 cores = 4 batches x 2 parities. Each core owns 4 query
blocks of 512 rows. Parity-1 cores receive x^T rolled left by 512
columns so every core's query blocks sit at uniform offsets 1024*i,
keeping the program SPMD-identical; causality is enforced by 4
data-driven diagonal mask tiles plus a parity-dependent pre-exp bias
(-1e30 kills the wrap-around key tiles on parity-0 cores).

Softmax uses no max-subtraction (scores ~N(0,0.25^2), exp is safe) and
the denominator comes from a ones-column appended to V, so there are no
cross-partition reductions. All matmuls run as float32r (full PE rate
at N>=256 moving dim; rel err ~2e-4).

Schedule: all x^T column-block DMAs are issued up-front (SP queue runs
them back-to-back at full DMA rate) in stream order 0,2,4,6,7,1,3,5;
blocks arrive in 1-2 chunk pieces so projection matmuls trickle behind
the DMA, interleaved with attention segments whose inputs are already
resident (PE is in-order, so emission order == data-arrival order).
Attention segments pipeline the S matmuls 2 key-tiles ahead of the AV
accumulation, across piece boundaries. PSUM banks: pkv 1 + shared
{pq,transpose} 2 + S 3 + attention-out 2 = 8.
"""

import numpy as np

B, T, D, H = 4, 4096, 1024, 64
NCORES = 8
QB = 512          # query block width (free dim of attention matmuls)
KT = 128          # key tile (partition dim of P^T)
DC = D // 128     # 8 contraction chunks
CB = 512          # x^T column block for streaming
NCB = T // CB     # 8
NKT = T // KT     # 32
NB = 4            # query blocks per core
HE = H + 1        # V extended with a ones column (softmax denominator)

_PROGRAM = None


def _build_program():
    from contextlib import ExitStack

    import concourse.bass as bass  # noqa: F401
    import concourse.mybir as mybir
    import concourse.tile as tile
    from concourse import bacc
    from concourse.masks import make_identity

    f32 = mybir.dt.float32
    f32r = mybir.dt.float32r
    AF = mybir.ActivationFunctionType

    nc = bacc.Bacc(target_bir_lowering=False)
    xt_d = nc.dram_tensor("xt", [D, T], f32r, kind="ExternalInput").ap()
    wq_d = nc.dram_tensor("wq", [128, DC * H], f32r, kind="ExternalInput").ap()
    wkv_d = nc.dram_tensor("wkv", [D, 2 * H], f32r, kind="ExternalInput").ap()
    bq_d = nc.dram_tensor("bq", [H, 1], f32, kind="ExternalInput").ap()
    bkv_d = nc.dram_tensor("bkv", [2 * H, 1], f32, kind="ExternalInput").ap()
    tb_d = nc.dram_tensor("tb", [KT, 1], f32, kind="ExternalInput").ap()
    on_d = nc.dram_tensor("ones", [128, NKT], f32r, kind="ExternalInput").ap()
    o_d = nc.dram_tensor("o", [NB * 2, 128, 2 * H], f32, kind="ExternalOutput").ap()

    with ExitStack() as ctx:
        tc = ctx.enter_context(tile.TileContext(nc))
        const = ctx.enter_context(tc.tile_pool(name="const", bufs=1))
        xt_pool = ctx.enter_context(tc.tile_pool(name="xtp", bufs=5))
        ppool = ctx.enter_context(tc.tile_pool(name="ptp", bufs=6))
        opool = ctx.enter_context(tc.tile_pool(name="otp", bufs=2))
        ps_a = ctx.enter_context(tc.tile_pool(name="psA", bufs=1, space="PSUM"))
        ps_qt = ctx.enter_context(tc.tile_pool(name="psQT", bufs=2, space="PSUM"))
        ps_s = ctx.enter_context(tc.tile_pool(name="psS", bufs=3, space="PSUM"))
        ps_o = ctx.enter_context(tc.tile_pool(name="psO", bufs=2, space="PSUM"))

        # Persistent SBUF state
        wq_s = const.tile([128, DC * H], f32r)        # chunk d at cols d*H
        wkv_s = const.tile([128, DC * 2 * H], f32r)   # chunk d at cols d*2H
        bq_s = const.tile([H, 1], f32)
        bkv_s = const.tile([2 * H, 1], f32)
        mk_s = const.tile([KT, 4 * QB], f32)          # mask slot s at cols s*QB
        tb_s = const.tile([KT, 1], f32)              # tail-tile exp bias
        zb_s = const.tile([KT, 1], f32)              # zero exp bias
        nc.vector.memset(zb_s, 0.0)
        ident = const.tile([128, 128], f32)
        kv_s = const.tile([128, T], f32r)             # rows 0:64 k^T, 64:128 v^T
        ve_s = const.tile([128, NKT * HE], f32r)      # key tile j at cols j*HE
        qt_s = const.tile([H, NB * QB], f32r)         # q^T, block i at cols i*QB

        make_identity(nc, ident)
        # Diagonal causal masks, generated on-chip: slot s keeps column c
        # of key-partition p iff c >= p + 128*s. GPSIMD is otherwise idle.
        nc.gpsimd.memset(mk_s, 1.0)
        for s in range(4):
            nc.gpsimd.affine_select(
                out=mk_s[:, s * QB:(s + 1) * QB],
                in_=mk_s[:, s * QB:(s + 1) * QB],
                compare_op=mybir.AluOpType.is_ge,
                fill=0.0,
                base=-128 * s,
                channel_multiplier=-1,
                pattern=[[1, QB]],
            )

        # ---- DMA issue (SP queue order == priority order) ----
        STREAM = [0, 2, 4, 6, 7, 1, 3, 5]
        xt_tiles = {}

        def issue_xt_dma(t, groups, lo=0, hi=None):
            """Issue chunk-range [lo, hi) of col-block t as `groups`-sized
            chunk-group DMAs."""
            if t not in xt_tiles:
                xt_tiles[t] = xt_pool.tile(
                    [128, DC * CB], f32r, tag="xt", name=f"xt{t}"
                )
            xt_t = xt_tiles[t]
            step = DC // groups
            for g in range(lo, hi if hi is not None else DC, step):
                nc.sync.dma_start(
                    out=xt_t.rearrange("p (d c) -> p d c", d=DC)[
                        :, g:g + step, :
                    ],
                    in_=xt_d.rearrange("(d p) t -> p d t", p=128)[
                        :, g:g + step, t * CB:(t + 1) * CB
                    ],
                )

        def issue_wkv_chunk(d0, d1):
            nc.sync.dma_start(
                out=wkv_s.rearrange("p (d h) -> p d h", d=DC)[:, d0:d1, :],
                in_=wkv_d.rearrange("(d p) h -> p d h", p=128)[:, d0:d1, :],
            )

        # Interleave the first weight chunk with the first x chunk so the
        # d-th KV matmul's inputs both land ~d*728ns in.
        issue_wkv_chunk(0, 1)
        issue_xt_dma(0, 8, lo=0, hi=1)
        issue_wkv_chunk(1, DC)
        nc.sync.dma_start(out=bq_s, in_=bq_d)
        nc.sync.dma_start(out=bkv_s, in_=bkv_d)
        nc.sync.dma_start(out=tb_s, in_=tb_d)
        issue_xt_dma(0, 8, lo=1)
        nc.sync.dma_start(out=wq_s, in_=wq_d)  # host pre-laid-out [128, DC*H]
        # Ones column of extended V (softmax denominator), strided into
        # every key tile's column H. memset can't target f32r tiles, so
        # the ones come from a tiny DRAM input.
        nc.sync.dma_start(
            out=ve_s.rearrange("p (j e) -> p j e", e=HE)[:, :, H:H + 1],
            in_=on_d.rearrange("p (j e) -> p j e", e=1),
        )
        for t in STREAM[1:]:
            issue_xt_dma(t, 4)

        # ---- compute emission, in data-arrival order ----

        def stage_mm(t):
            """Projection matmuls + bias drains + v-transposes for block t."""
            xt_t = xt_tiles[t]
            pkv = ps_a.tile([128, CB], f32, tag="pkv")
            for d in range(DC):
                nc.tensor.matmul(
                    pkv,
                    lhsT=wkv_s[:, d * 128:(d + 1) * 128],
                    rhs=xt_t[:, d * CB:(d + 1) * CB],
                    start=(d == 0),
                    stop=(d == DC - 1),
                )
            pq = None
            if t % 2 == 0:
                pq = ps_qt.tile([128, CB], f32, tag="qt", name="pq")
                for d in range(DC):
                    nc.tensor.matmul(
                        pq[0:H, :],
                        lhsT=wq_s[:, d * H:(d + 1) * H],
                        rhs=xt_t[:, d * CB:(d + 1) * CB],
                        start=(d == 0),
                        stop=(d == DC - 1),
                    )
            nc.vector.tensor_scalar_add(
                kv_s[:, t * CB:(t + 1) * CB], pkv, bkv_s
            )
            if pq is not None:
                i = t // 2
                nc.vector.tensor_scalar_add(
                    qt_s[:, i * QB:(i + 1) * QB], pq[0:H, :], bq_s
                )
            for sub in range(4):                     # v^T -> natural-v tiles
                j = 4 * t + sub
                ptr = ps_qt.tile([128, CB], f32, tag="qt", name="ptr")
                nc.tensor.transpose(
                    ptr[:, 0:H],
                    kv_s[64:128, t * CB + sub * KT:t * CB + (sub + 1) * KT].bitcast(f32),
                    ident[64:128, 64:128],
                )
                nc.vector.tensor_copy(ve_s[:, j * HE:j * HE + H], ptr[:, 0:H])

        # Per-block SBUF accumulators for (PV | denom)^T; pieces of a
        # block's key loop flush their PSUM partial here so attention can
        # be emitted piecewise as kv col-blocks arrive.
        oacc = []
        for _i in range(NB):
            acc_tile = const.tile([HE, QB], f32, tag=f"oacc{_i}")
            oacc.append(acc_tile)
        first_piece = [True] * NB

        LOOK = 2

        def attn_seg(pieces):
            """Emit S->exp->mask->AV for a list of (i, js) pieces whose kv/q
            inputs are all resident. The S matmul runs LOOK key-tiles ahead
            of the AV accumulation, across piece boundaries, so the PE does
            not stall on the ScalarE exp latency. Each piece accumulates in
            its own PSUM bank (2 rotating) and drains into the block's SBUF
            accumulator."""
            tiles = []                       # (piece_idx, i, j)
            for pi, (i, js) in enumerate(pieces):
                for j in js:
                    tiles.append((pi, i, j))
            pos = {pi: ps_o.tile([HE, QB], f32, tag="po", name=f"po{pi}")
                   for pi in range(len(pieces))}
            last_of = {}
            for k, (pi, _i, _j) in enumerate(tiles):
                last_of[pi] = k
            pts = {}

            def emit_s(k):
                pi, i, j = tiles[k]
                ps = ps_s.tile([KT, QB], f32)
                nc.tensor.matmul(
                    ps,
                    lhsT=kv_s[0:64, j * KT:(j + 1) * KT],
                    rhs=qt_s[:, i * QB:(i + 1) * QB],
                    start=True,
                    stop=True,
                )
                pt = ppool.tile([KT, QB], f32r)
                # Tail (wrap-around) tiles: parity-0 cores kill them with a
                # -1e30 pre-exp bias; parity-1 keeps them (bias 0).
                bias = tb_s if j >= 28 else zb_s
                nc.scalar.activation(
                    pt, ps, AF.Exp, bias=bias, scale=float(D) ** -0.5
                )
                if 8 * i <= j < 8 * i + 4:
                    slot = j - 8 * i                 # diagonal masks
                    nc.vector.tensor_mul(
                        pt, pt,
                        mk_s[:, slot * QB:(slot + 1) * QB].bitcast(f32r)
                    )
                pts[k] = pt

            n = len(tiles)
            for k in range(min(LOOK, n)):
                emit_s(k)
            starts = {pi: True for pi in range(len(pieces))}
            for k in range(n):
                if k + LOOK < n:
                    emit_s(k + LOOK)
                pi, i, j = tiles[k]
                nc.tensor.matmul(
                    pos[pi],
                    lhsT=ve_s[:, j * HE:(j + 1) * HE],
                    rhs=pts.pop(k),
                    start=starts[pi],
                    stop=(k == last_of[pi]),
                )
                starts[pi] = False
                if k == last_of[pi]:
                    if first_piece[i]:
                        nc.vector.tensor_copy(oacc[i], pos[pi])
                        first_piece[i] = False
                    else:
                        nc.vector.tensor_add(oacc[i], oacc[i], pos[pi])

        def attn_epi(i):
            for sub in range(4):
                ptr = ps_qt.tile([128, CB], f32, tag="qt", name="eptr")
                nc.tensor.transpose(
                    ptr[:, 0:HE], oacc[i][:, sub * 128:(sub + 1) * 128],
                    ident[0:HE, 0:HE]
                )
                rcp = opool.tile([128, 1], f32, tag="rcp")
                nc.vector.reciprocal(rcp, ptr[:, H:H + 1])
                if sub % 2 == 0:
                    ot2 = opool.tile([128, 2 * H], f32, tag="out")
                nc.vector.tensor_scalar_mul(
                    ot2[:, (sub % 2) * H:(sub % 2 + 1) * H], ptr[:, 0:H], rcp)
                if sub % 2 == 1:
                    nc.sync.dma_start(
                        out=o_d[i * 2 + (sub - 1) // 2], in_=ot2,
                    )

        # Emission order mirrors DMA arrival: per col-block, the trickled
        # projections come first, then every attention piece whose kv/q
        # inputs became resident with this block. Diag piece for block i
        # needs col 2i (q + diag kv); tails need col 7 + q_i; full tiles
        # j need col j//4 + q_i.
        stage_mm(0)
        attn_seg([(0, [0, 1, 2, 3])])               # diag0
        stage_mm(2)
        attn_seg([(1, [8, 9, 10, 11])])             # diag1
        stage_mm(4)
        attn_seg([(1, [0, 1, 2, 3]),
                  (2, [16, 17, 18, 19])])           # diag2
        stage_mm(6)
        attn_seg([(2, [0, 1, 2, 3, 8, 9, 10, 11]),
                  (3, [24, 25, 26, 27])])           # diag3
        stage_mm(7)
        attn_seg([(0, [28, 29, 30, 31]),            # tails b0
                  (3, [0, 1, 2, 3, 8, 9, 10, 11, 16, 17, 18, 19])])
        attn_epi(0)
        stage_mm(1)
        attn_seg([(1, [28, 29, 30, 31, 4, 5, 6, 7]),
                  (2, [28, 29, 30, 31, 4, 5, 6, 7])])
        attn_epi(1)
        stage_mm(3)
        attn_seg([(2, [12, 13, 14, 15]),
                  (3, [28, 29, 30, 31, 4, 5, 6, 7, 12, 13, 14, 15])])
        attn_epi(2)
        stage_mm(5)
        attn_seg([(3, [20, 21, 22, 23])])
        attn_epi(3)

    nc.compile()
    return nc


def _get_program():
    global _PROGRAM
    if _PROGRAM is None:
        _PROGRAM = _build_program()
    return _PROGRAM


def build_in_maps(inputs):
    x = np.asarray(inputs["x"], np.float32)
    wq = np.asarray(inputs["Wq"], np.float32)            # [D, H]
    # host re-layout to [128, DC*H]: chunk d (rows d*128..) at cols d*H
    wq = np.ascontiguousarray(
        wq.reshape(DC, 128, H).transpose(1, 0, 2).reshape(128, DC * H)
    )
    wkv = np.ascontiguousarray(
        np.concatenate(
            [np.asarray(inputs["Wk"], np.float32),
             np.asarray(inputs["Wv"], np.float32)], axis=1
        )
    )
    bq = np.ascontiguousarray(np.asarray(inputs["bq"], np.float32)[:, None])
    bkv = np.ascontiguousarray(
        np.concatenate(
            [np.asarray(inputs["bk"], np.float32),
             np.asarray(inputs["bv"], np.float32)]
        )[:, None]
    )
    in_maps = []
    for core in range(NCORES):
        b, p = core // 2, core % 2
        xt = x[b].T
        if p:
            xt = np.roll(xt, -512, axis=1)
        in_maps.append({
            "xt": np.ascontiguousarray(xt),
            "wq": wq,
            "wkv": wkv,
            "bq": bq,
            "bkv": bkv,
            "ones": np.ones((128, NKT), np.float32),
            "tb": np.full((KT, 1), 0.0 if p == 1 else -1e30, np.float32),
        })
    return in_maps


def assemble_out(results):
    out = np.empty((B, T, H), np.float32)
    for core in range(NCORES):
        b, p = core // 2, core % 2
        o = np.asarray(results[core]["o"])    # [NB*2, 128, 2H]
        o = o.reshape(NB * 2, 128, 2, H).transpose(0, 2, 1, 3).reshape(NB, QB, H)
        for i in range(NB):
            g = 1024 * i + 512 * p
            out[b, g:g + QB] = o[i]
    return out


def kernel(**inputs):
    from concourse.bass_utils import run_bass_kernel_spmd

    nc = _get_program()
    in_maps = build_in_maps(inputs)
    res = run_bass_kernel_spmd(nc, in_maps, list(range(NCORES)))
    return assemble_out(res.results)


# revision 21
# speedup vs baseline: 1.0197x; 1.0197x over previous
"""Causal single-head attention on 8 TRN2 NeuronCores (Bass/Tile).

Problem: x[4,4096,1024] @ {Wq,Wk,Wv}[1024,64] (+zero biases) -> causal
softmax attention -> out[4,4096,64], fp32.

Sharding: 8 cores = 4 batches x 2 parities. Each core owns 4 query
blocks of 512 rows. Parity-1 cores receive x^T rolled left by 512
columns so every core's query blocks sit at uniform offsets 1024*i,
keeping the program SPMD-identical; causality is enforced by 4
data-driven diagonal mask tiles plus a parity-dependent pre-exp bias
(-1e30 kills the wrap-around key tiles on parity-0 cores).

Softmax uses no max-subtraction (scores ~N(0,0.25^2), exp is safe) and
the denominator comes from a ones-column appended to V, so there are no
cross-partition reductions. All matmuls run as float32r (full PE rate
at N>=256 moving dim; rel err ~2e-4).

Schedule: all x^T column-block DMAs are issued up-front (SP queue runs
them back-to-back at full DMA rate) in stream order 0,2,4,6,7,1,3,5;
blocks arrive in 1-2 chunk pieces so projection matmuls trickle behind
the DMA, interleaved with attention segments whose inputs are already
resident (PE is in-order, so emission order == data-arrival order).
Attention segments pipeline the S matmuls 2 key-tiles ahead of the AV
accumulation, across piece boundaries. PSUM banks: pkv 1 + shared
{pq,transpose} 2 + S 3 + attention-out 2 = 8.
"""

import numpy as np

B, T, D, H = 4, 4096, 1024, 64
NCORES = 8
QB = 512          # query block width (free dim of attention matmuls)
KT = 128          # key tile (partition dim of P^T)
DC = D // 128     # 8 contraction chunks
CB = 512          # x^T column block for streaming
NCB = T // CB     # 8
NKT = T // KT     # 32
NB = 4            # query blocks per core
HE = H + 1        # V extended with a ones column (softmax denominator)

_PROGRAM = None


def _build_program():
    from contextlib import ExitStack

    import concourse.bass as bass  # noqa: F401
    import concourse.mybir as mybir
    import concourse.tile as tile
    from concourse import bacc
    from concourse.masks import make_identity

    f32 = mybir.dt.float32
    f32r = mybir.dt.float32r
    AF = mybir.ActivationFunctionType

    nc = bacc.Bacc(target_bir_lowering=False)
    xt_d = nc.dram_tensor("xt", [D, T], f32r, kind="ExternalInput").ap()
    wq_d = nc.dram_tensor("wq", [128, DC * H], f32r, kind="ExternalInput").ap()
    wkv_d = nc.dram_tensor("wkv", [D, 2 * H], f32r, kind="ExternalInput").ap()
    # col 0 rows 0:64 = bq, col 1 = bkv, col 2 = tb — one tiny load
    bias_d = nc.dram_tensor("bias", [128, 3], f32, kind="ExternalInput").ap()
    on_d = nc.dram_tensor("ones", [128, NKT], f32r, kind="ExternalInput").ap()
    o_d = nc.dram_tensor("o", [NB * 2, 128, 2 * H], f32, kind="ExternalOutput").ap()

    with ExitStack() as ctx:
        tc = ctx.enter_context(tile.TileContext(nc))
        const = ctx.enter_context(tc.tile_pool(name="const", bufs=1))
        xt_pool = ctx.enter_context(tc.tile_pool(name="xtp", bufs=5))
        ppool = ctx.enter_context(tc.tile_pool(name="ptp", bufs=10))
        opool = ctx.enter_context(tc.tile_pool(name="otp", bufs=2))
        ps_a = ctx.enter_context(tc.tile_pool(name="psA", bufs=1, space="PSUM"))
        ps_qt = ctx.enter_context(tc.tile_pool(name="psQT", bufs=2, space="PSUM"))
        ps_s = ctx.enter_context(tc.tile_pool(name="psS", bufs=3, space="PSUM"))
        ps_o = ctx.enter_context(tc.tile_pool(name="psO", bufs=2, space="PSUM"))

        # Persistent SBUF state
        wq_s = const.tile([128, DC * H], f32r)        # chunk d at cols d*H
        wkv_s = const.tile([128, DC * 2 * H], f32r)   # chunk d at cols d*2H
        bias_s = const.tile([128, 3], f32)
        bq_s = bias_s[0:H, 0:1]
        bkv_s = bias_s[:, 1:2]
        tb_s = bias_s[:, 2:3]                        # tail-tile exp bias
        mk_s = const.tile([KT, 4 * QB], f32)          # mask slot s at cols s*QB
        zb_s = const.tile([KT, 1], f32)              # zero exp bias
        nc.vector.memset(zb_s, 0.0)
        ident = const.tile([128, 128], f32)
        kv_s = const.tile([128, T], f32r)             # rows 0:64 k^T, 64:128 v^T
        ve_s = const.tile([128, NKT * HE], f32r)      # key tile j at cols j*HE
        qt_s = const.tile([H, NB * QB], f32r)         # q^T, block i at cols i*QB

        make_identity(nc, ident)
        # Diagonal causal masks, generated on-chip: slot s keeps column c
        # of key-partition p iff c >= p + 128*s. GPSIMD is otherwise idle.
        nc.gpsimd.memset(mk_s, 1.0)
        for s in range(4):
            nc.gpsimd.affine_select(
                out=mk_s[:, s * QB:(s + 1) * QB],
                in_=mk_s[:, s * QB:(s + 1) * QB],
                compare_op=mybir.AluOpType.is_ge,
                fill=0.0,
                base=-128 * s,
                channel_multiplier=-1,
                pattern=[[1, QB]],
            )

        # ---- DMA issue (SP queue order == priority order) ----
        STREAM = [0, 2, 4, 6, 7, 1, 3, 5]
        xt_tiles = {}

        def issue_xt_dma(t, groups, lo=0, hi=None):
            """Issue chunk-range [lo, hi) of col-block t as `groups`-sized
            chunk-group DMAs."""
            if t not in xt_tiles:
                xt_tiles[t] = xt_pool.tile(
                    [128, DC * CB], f32r, tag="xt", name=f"xt{t}"
                )
            xt_t = xt_tiles[t]
            step = DC // groups
            for g in range(lo, hi if hi is not None else DC, step):
                nc.sync.dma_start(
                    out=xt_t.rearrange("p (d c) -> p d c", d=DC)[
                        :, g:g + step, :
                    ],
                    in_=xt_d.rearrange("(d p) t -> p d t", p=128)[
                        :, g:g + step, t * CB:(t + 1) * CB
                    ],
                )

        def issue_wkv_chunk(d0, d1):
            nc.sync.dma_start(
                out=wkv_s.rearrange("p (d h) -> p d h", d=DC)[:, d0:d1, :],
                in_=wkv_d.rearrange("(d p) h -> p d h", p=128)[:, d0:d1, :],
            )

        # Interleave the first weight chunk with the first x chunk so the
        # d-th KV matmul's inputs both land ~d*728ns in.
        issue_wkv_chunk(0, 1)
        issue_xt_dma(0, 8, lo=0, hi=1)
        nc.sync.dma_start(out=wq_s, in_=wq_d)  # host pre-laid-out [128, DC*H]
        issue_wkv_chunk(1, DC)
        issue_xt_dma(0, 8, lo=1, hi=2)
        nc.sync.dma_start(out=bias_s, in_=bias_d)
        issue_xt_dma(0, 8, lo=2)
        # Ones column of extended V (softmax denominator), strided into
        # every key tile's column H. memset can't target f32r tiles, so
        # the ones come from a tiny DRAM input.
        nc.sync.dma_start(
            out=ve_s.rearrange("p (j e) -> p j e", e=HE)[:, :, H:H + 1],
            in_=on_d.rearrange("p (j e) -> p j e", e=1),
        )
        for t in STREAM[1:]:
            issue_xt_dma(t, 4)

        # ---- compute emission, in data-arrival order ----

        def stage_mm(t):
            """Projection matmuls + bias drains + v-transposes for block t.
            KV and Q matmuls alternate per contraction chunk so the PE
            trickles behind the chunk DMAs with no program-order hazard."""
            xt_t = xt_tiles[t]
            pkv = ps_a.tile([128, CB], f32, tag="pkv")
            pq = None
            if t % 2 == 0:
                pq = ps_qt.tile([128, CB], f32, tag="qt", name="pq")
            for d in range(DC):
                nc.tensor.matmul(
                    pkv,
                    lhsT=wkv_s[:, d * 128:(d + 1) * 128],
                    rhs=xt_t[:, d * CB:(d + 1) * CB],
                    start=(d == 0),
                    stop=(d == DC - 1),
                )
                if pq is not None:
                    nc.tensor.matmul(
                        pq[0:H, :],
                        lhsT=wq_s[:, d * H:(d + 1) * H],
                        rhs=xt_t[:, d * CB:(d + 1) * CB],
                        start=(d == 0),
                        stop=(d == DC - 1),
                    )
            nc.vector.tensor_scalar_add(
                kv_s[:, t * CB:(t + 1) * CB], pkv, bkv_s
            )
            if pq is not None:
                i = t // 2
                nc.vector.tensor_scalar_add(
                    qt_s[:, i * QB:(i + 1) * QB], pq[0:H, :], bq_s
                )
            for sub in range(4):                     # v^T -> natural-v tiles
                j = 4 * t + sub
                ptr = ps_qt.tile([128, CB], f32, tag="qt", name="ptr")
                nc.tensor.transpose(
                    ptr[:, 0:H],
                    kv_s[64:128, t * CB + sub * KT:t * CB + (sub + 1) * KT].bitcast(f32),
                    ident[64:128, 64:128],
                )
                nc.vector.tensor_copy(ve_s[:, j * HE:j * HE + H], ptr[:, 0:H])

        # Per-block SBUF accumulators for (PV | denom)^T; pieces of a
        # block's key loop flush their PSUM partial here so attention can
        # be emitted piecewise as kv col-blocks arrive.
        oacc = []
        for _i in range(NB):
            acc_tile = const.tile([HE, QB], f32, tag=f"oacc{_i}")
            oacc.append(acc_tile)
        first_drain = [True] * NB

        LOOK = 3
        pending = {i: [] for i in range(NB)}     # block -> [(j, pt), ...]

        def s_tile(i, j):
            """Emit S -> exp (-> diag mask); result pt goes to pending[i]."""
            ps = ps_s.tile([KT, QB], f32)
            nc.tensor.matmul(
                ps,
                lhsT=kv_s[0:64, j * KT:(j + 1) * KT],
                rhs=qt_s[:, i * QB:(i + 1) * QB],
                start=True,
                stop=True,
            )
            pt = ppool.tile([KT, QB], f32r)
            # Tail (wrap-around) tiles: parity-0 cores kill them with a
            # -1e30 pre-exp bias; parity-1 keeps them (bias 0).
            bias = tb_s if j >= 28 else zb_s
            nc.scalar.activation(
                pt, ps, AF.Exp, bias=bias, scale=float(D) ** -0.5
            )
            if 8 * i <= j < 8 * i + 4:
                slot = j - 8 * i                     # diagonal masks
                nc.vector.tensor_mul(
                    pt, pt,
                    mk_s[:, slot * QB:(slot + 1) * QB].bitcast(f32r)
                )
            pending[i].append((j, pt))

        def attn_phase(ops):
            """Emit one phase. `ops` is an ordered list of:
              ("blk", i, js, drain) -- AV all pending tiles of block i plus
                  the new tiles `js` (S-emitted within this phase, LOOK
                  tiles ahead of their AV), as one PSUM accumulation group;
                  drain to oacc[i] at the end if requested.
              ("pre", i, js) -- S+exp(+mask) only; AVs happen in a later
                  phase (gives diag mask-muls a whole phase of cover).
            """
            # Build the phase-global S stream and AV stream.
            s_stream = []                            # (i, j)
            av_stream = []                           # (i, kind, payload)
            for op in ops:
                if op[0] == "pre":
                    _, i, js = op
                    s_stream.extend((i, j) for j in js)
                else:
                    _, i, js, drain = op
                    n_pend = len(pending[i])
                    s_stream.extend((i, j) for j in js)
                    av_stream.append((i, n_pend + len(js), n_pend, drain))
            # Greedy interleave: keep the S emission LOOK tiles ahead of
            # new-tile AV consumption; pending tiles are already covered.
            s_ptr = 0
            consumed_new = 0                          # AV'd tiles that were S'd this phase
            for i, n_tiles, n_pend0, drain in av_stream:
                po = ps_o.tile([HE, QB], f32, tag="po", name=f"po{i}")
                for t in range(n_tiles):
                    if t >= n_pend0:
                        consumed_new += 1
                    while s_ptr < len(s_stream) and (
                        s_ptr < consumed_new + LOOK
                    ):
                        s_tile(*s_stream[s_ptr])
                        s_ptr += 1
                    while not pending[i]:             # force-emit if starved
                        s_tile(*s_stream[s_ptr])
                        s_ptr += 1
                    j, pt = pending[i].pop(0)
                    nc.tensor.matmul(
                        po,
                        lhsT=ve_s[:, j * HE:(j + 1) * HE],
                        rhs=pt,
                        start=(t == 0),
                        stop=(t == n_tiles - 1),
                    )
                if drain:
                    if first_drain[i]:
                        nc.vector.tensor_copy(oacc[i], po)
                        first_drain[i] = False
                    else:
                        nc.vector.tensor_add(oacc[i], oacc[i], po)
            # Flush any S-only tails (pre ops with no following AVs).
            while s_ptr < len(s_stream):
                s_tile(*s_stream[s_ptr])
                s_ptr += 1

        def attn_epi(i):
            for sub in range(4):
                ptr = ps_qt.tile([128, CB], f32, tag="qt", name="eptr")
                nc.tensor.transpose(
                    ptr[:, 0:HE], oacc[i][:, sub * 128:(sub + 1) * 128],
                    ident[0:HE, 0:HE]
                )
                rcp = opool.tile([128, 1], f32, tag="rcp")
                nc.vector.reciprocal(rcp, ptr[:, H:H + 1])
                if sub % 2 == 0:
                    ot2 = opool.tile([128, 2 * H], f32, tag="out")
                nc.vector.tensor_scalar_mul(
                    ot2[:, (sub % 2) * H:(sub % 2 + 1) * H], ptr[:, 0:H], rcp)
                if sub % 2 == 1:
                    nc.sync.dma_start(
                        out=o_d[i * 2 + (sub - 1) // 2], in_=ot2,
                    )

        # Phase layout: each attn_phase consumes data from cols loaded in
        # earlier windows and runs during the next col's DMA window. Diag
        # pieces are S-emitted one phase early ("pre") so their DVE
        # mask-multiply latency is covered; their AVs land next phase.
        stage_mm(0)
        attn_phase([("pre", 0, [0, 1, 2, 3])])
        stage_mm(2)
        attn_phase([("blk", 0, [], True),
                    ("blk", 1, [0, 1, 2, 3], True),
                    ("pre", 1, [8, 9, 10, 11])])
        stage_mm(4)
        attn_phase([("blk", 1, [], True),
                    ("blk", 2, [0, 1, 2, 3, 8, 9, 10, 11], True),
                    ("pre", 2, [16, 17, 18, 19])])
        stage_mm(6)
        attn_phase([("blk", 2, [], True),
                    ("blk", 3, [0, 1, 2, 3], True),
                    ("pre", 3, [24, 25, 26, 27])])
        stage_mm(7)
        attn_phase([("blk", 3, [8, 9, 10, 11, 16, 17, 18, 19], True),
                    ("blk", 0, [28, 29, 30, 31], True)])
        attn_epi(0)
        stage_mm(1)
        attn_phase([("blk", 1, [28, 29, 30, 31, 4, 5, 6, 7], True),
                    ("blk", 2, [28, 29, 30, 31, 4, 5, 6, 7], True)])
        attn_epi(1)
        stage_mm(3)
        attn_phase([("blk", 2, [12, 13, 14, 15], True),
                    ("blk", 3, [28, 29, 30, 31, 4, 5, 6, 7,
                                12, 13, 14, 15], True)])
        attn_epi(2)
        stage_mm(5)
        attn_phase([("blk", 3, [20, 21, 22, 23], True)])
        attn_epi(3)

    nc.compile()
    return nc


def _get_program():
    global _PROGRAM
    if _PROGRAM is None:
        _PROGRAM = _build_program()
    return _PROGRAM


def build_in_maps(inputs):
    x = np.asarray(inputs["x"], np.float32)
    wq = np.asarray(inputs["Wq"], np.float32)            # [D, H]
    # host re-layout to [128, DC*H]: chunk d (rows d*128..) at cols d*H
    wq = np.ascontiguousarray(
        wq.reshape(DC, 128, H).transpose(1, 0, 2).reshape(128, DC * H)
    )
    wkv = np.ascontiguousarray(
        np.concatenate(
            [np.asarray(inputs["Wk"], np.float32),
             np.asarray(inputs["Wv"], np.float32)], axis=1
        )
    )
    bias = np.zeros((2, 128, 3), np.float32)
    for p in range(2):
        bias[p, 0:H, 0] = np.asarray(inputs["bq"], np.float32)
        bias[p, 0:H, 1] = np.asarray(inputs["bk"], np.float32)
        bias[p, H:2 * H, 1] = np.asarray(inputs["bv"], np.float32)
        bias[p, :, 2] = 0.0 if p == 1 else -1e30
    in_maps = []
    for core in range(NCORES):
        b, p = core // 2, core % 2
        xt = x[b].T
        if p:
            xt = np.roll(xt, -512, axis=1)
        in_maps.append({
            "xt": np.ascontiguousarray(xt),
            "wq": wq,
            "wkv": wkv,
            "bias": np.ascontiguousarray(bias[p]),
            "ones": np.ones((128, NKT), np.float32),
        })
    return in_maps


def assemble_out(results):
    out = np.empty((B, T, H), np.float32)
    for core in range(NCORES):
        b, p = core // 2, core % 2
        o = np.asarray(results[core]["o"])    # [NB*2, 128, 2H]
        o = o.reshape(NB * 2, 128, 2, H).transpose(0, 2, 1, 3).reshape(NB, QB, H)
        for i in range(NB):
            g = 1024 * i + 512 * p
            out[b, g:g + QB] = o[i]
    return out


def kernel(**inputs):
    from concourse.bass_utils import run_bass_kernel_spmd

    nc = _get_program()
    in_maps = build_in_maps(inputs)
    res = run_bass_kernel_spmd(nc, in_maps, list(range(NCORES)))
    return assemble_out(res.results)
